# revision 1
# baseline (speedup 1.0000x reference)
"""Self-attention kernel for Trainium2 (8 NeuronCores, SPMD).

Problem: X[8192,512], Wq,Wk[512,512]:
    Q = X@Wq ; K = X@Wk ; S = softmax(Q K^T / sqrt(512)) ; out = S @ X

Sharding: rows of Q (query blocks of 1024) across 8 cores; K/V (=X) replicated.

Per-core dataflow (core owns query rows i in [c*1024, (c+1)*1024)):
  Phase P:  Q^T = (Wq^T X_mine^T)/sqrt(d)   [512,1024]   (resident, f32r)
            K^T = Wk^T X^T                  [512,8192]   (spilled to DRAM, f32r)
  Per i-half h (512 query columns):
    B1: for each j-tile (64): S^T tile [j=128, i=512] = sum_d K^T_tile.T Q^T
        (4 accumulating f32r matmuls) -> ACT copy PSUM->SBUF (S region),
        DVE running elementwise max -> mx[128,512]
    B2: partition-reduce mx via PE transpose + DVE reduce_max -> m[1,512];
        broadcast back to [128,512] via ones outer-product matmul
    B3: for each j-tile: d = S^T_t - B (DVE), clamp(-80) (DVE), exp (ACT,
        f32r out) -> P~; PE: 4 accumulating matmuls o[i-chunk,512v] +=
        P~[:,chunk].T @ X[j-tile] and 1 ones-matmul sum[1,512] += col-sums
    B4: recip(sum), transpose to per-partition cols, ACT Copy-with-scale
        drains o PSUM -> SBUF, DMA out.

The host supplies X, X^T and the per-core X^T slice as separate inputs
(layout staging only; all FLOPs happen on device). fp32r matmuls keep
~13 mantissa bits => logit noise ~0.08 => output rel err ~3e-3.
"""
import sys

sys.path.insert(0, "/opt/trn_rl_repo")

import numpy as np

import concourse.bass as bass
import concourse.mybir as mybir
import concourse.tile as tile
from concourse import bacc
from concourse.bass import ts
from concourse.bass_utils import run_bass_kernel_spmd
from concourse.masks import make_identity

F32 = mybir.dt.float32
F32R = mybir.dt.float32r
F16 = mybir.dt.float16
AF = mybir.ActivationFunctionType
ALU = mybir.AluOpType

N = 8192
D = 512
NCORES = 8
MY_N = N // NCORES          # 1024 query rows per core
NJT = N // 128              # 64 j-tiles
NIH = MY_N // 512           # 2 i-halves
CLAMP = -80.0

_NC_CACHE = None


def _build_nc():
    nc = bacc.Bacc(None, target_bir_lowering=False)

    xt = nc.dram_tensor("xt", [128, N // 512, 4, 512], F32R, kind="ExternalInput")  # X^T blocked
    xtm = nc.dram_tensor("xtm", [D, MY_N], F32R, kind="ExternalInput")   # X^T slice
    x = nc.dram_tensor("x", [128, N // 512, 4, 512], F16, kind="ExternalInput")  # X blocked fp16
    wq = nc.dram_tensor("wq", [D, D], F32R, kind="ExternalInput")
    wkt = nc.dram_tensor("wkt", [D, D], F32R, kind="ExternalInput")      # Wk^T
    o = nc.dram_tensor("o", [MY_N, D], F32, kind="ExternalOutput")

    with tile.TileContext(nc) as tc:
        with (
            tc.tile_pool(name="pool", bufs=1) as pool,          # persistent
            tc.tile_pool(name="stream", bufs=2) as stream,      # xt blocks
            tc.tile_pool(name="wpool", bufs=1) as wpool,        # wq then wkt
            tc.tile_pool(name="big", bufs=1) as big,            # xtm then S-region
            tc.tile_pool(name="xs", bufs=2) as xsp,             # X tiles (B3)
            tc.tile_pool(name="work", bufs=2) as work,          # d / p
            tc.tile_pool(name="osbp", bufs=1) as osbp,
            tc.tile_pool(name="ps_qk", bufs=3, space="PSUM") as ps_qk,
            tc.tile_pool(name="ps_o", bufs=1, space="PSUM") as ps_o,
            tc.tile_pool(name="ps_sum", bufs=1, space="PSUM") as ps_sum,
        ):
            # ---- constants ----
            ident = pool.tile([128, 128], F32)
            make_identity(nc, ident[:])
            ones_f32 = pool.tile([128, 2], F32)
            nc.vector.memset(ones_f32[:], 1.0)
            ones_col = pool.tile([128, 1], F16)    # lhsT for column sums
            nc.vector.tensor_copy(ones_col[:], ones_f32[:, 0:1])
            ones_row_f32 = pool.tile([1, 128], F32)
            nc.vector.memset(ones_row_f32[:], 1.0)
            ones_row = pool.tile([1, 128], F32R)   # lhsT for broadcast
            nc.vector.tensor_copy(ones_row[:], ones_row_f32[:])
            one_one = pool.tile([1, 1], F32)
            nc.vector.memset(one_one[:], 1.0)

            # ---- Phase P1: Q_s^T = (Wq^T X_mine^T) / sqrt(D) ----
            qt_sb = pool.tile([128, 4, MY_N], F32R, tag="qt")
            wq_sb = wpool.tile([128, 4, D], F32R, tag="w")
            nc.sync.dma_start(wq_sb[:], wq[:].rearrange("(c p) d -> p c d", p=128))
            xtm_sb = big.tile([128, 4, MY_N], F32R, tag="big")
            for ih in range(NIH):
                nc.sync.dma_start(
                    xtm_sb[:, :, ts(ih, 512)],
                    xtm[:, ts(ih, 512)].rearrange("(c p) i -> p c i", p=128),
                )
            scale = 1.0 / float(np.sqrt(D))
            for ih in range(NIH):
                for dch in range(4):
                    q_ps = ps_qk.tile([128, 512], F32, tag="qk")
                    for e in range(4):
                        nc.tensor.matmul(
                            q_ps[:],
                            wq_sb[:, e, ts(dch, 128)],
                            xtm_sb[:, e, ts(ih, 512)],
                            start=(e == 0),
                            stop=(e == 3),
                        )
                    nc.scalar.activation(
                        qt_sb[:, dch, ts(ih, 512)], q_ps[:], AF.Copy,
                        bias=0.0, scale=scale,
                    )

            # ---- Phase P2: R = Wk Q_s^T  (so S^T tiles = X^T-chunk.T @ R) ----
            r_sb = pool.tile([128, 4, MY_N], F32R, tag="r")
            wkt_sb = wpool.tile([128, 4, D], F32R, tag="w")
            nc.sync.dma_start(wkt_sb[:], wkt[:].rearrange("(c p) e -> p c e", p=128))
            for ih in range(NIH):
                for ech in range(4):
                    r_ps = ps_qk.tile([128, 512], F32, tag="qk")
                    for dch in range(4):
                        nc.tensor.matmul(
                            r_ps[:],
                            wkt_sb[:, dch, ts(ech, 128)],
                            qt_sb[:, dch, ts(ih, 512)],
                            start=(dch == 0),
                            stop=(dch == 3),
                        )
                    nc.scalar.copy(r_sb[:, ech, ts(ih, 512)], r_ps[:])

            def running_max(mx, s_ps, first):
                if first:
                    nc.vector.tensor_copy(mx[:], s_ps[:])
                else:
                    nc.vector.tensor_tensor(mx[:], mx[:], s_ps[:], op=ALU.max)

            def finalize_max(mx):
                """mx[128,512] -> b_sb[128,512] broadcast of per-i max."""
                mcol = pool.tile([128, 4], F32, tag="mcol")
                for c in range(4):
                    mt_ps = ps_qk.tile([128, 128], F32, tag="qk")
                    nc.tensor.transpose(mt_ps[:], mx[:, ts(c, 128)], ident[:])
                    nc.vector.reduce_max(
                        mcol[:, c : c + 1], mt_ps[:], axis=mybir.AxisListType.X
                    )
                mrow_ps = ps_qk.tile([1, 512], F32, tag="qk")
                for c in range(4):
                    nc.tensor.transpose(
                        mrow_ps[:, ts(c, 128)], mcol[:, c : c + 1], ident[:]
                    )
                mrow = pool.tile([1, 512], F32R, tag="mrow")
                nc.scalar.copy(mrow[:], mrow_ps[:])
                b_ps = ps_qk.tile([128, 512], F32, tag="qk")
                nc.tensor.matmul(b_ps[:], ones_row[:], mrow[:], start=True, stop=True)
                b_sb = pool.tile([128, 512], F32, tag="bsb")
                nc.scalar.copy(b_sb[:], b_ps[:])
                return b_sb

            def b1_qk(h, st, mx):
                for blk in range(N // 512):
                    xt_blk = stream.tile([128, 4, 512], F32R, tag="stream")
                    nc.sync.dma_start(xt_blk[:], xt[:, blk, :, :])
                    for t in range(4):
                        jt = blk * 4 + t
                        s_ps = ps_qk.tile([128, 512], F32, tag="qk")
                        for e in range(4):
                            nc.tensor.matmul(
                                s_ps[:],
                                xt_blk[:, e, ts(t, 128)],
                                r_sb[:, e, ts(h, 512)],
                                start=(e == 0),
                                stop=(e == 3),
                            )
                        nc.scalar.copy(st[:, jt, :], s_ps[:])
                        running_max(mx, s_ps, jt == 0)

            def b3_exp_and_accum(st, b_sb, o_ps, sum_ps):
                for jt in range(NJT):
                    if jt % 2 == 0:
                        x_blk = xsp.tile([128, 2, 512], F16, tag="x")
                        nc.sync.dma_start(
                            x_blk[:], x[:, jt // 4, jt % 4 : jt % 4 + 2, :]
                        )
                    x_t = x_blk[:, jt % 2, :]
                    d_t = work.tile([128, 512], F32, tag="d")
                    nc.vector.tensor_tensor(
                        d_t[:], st[:, jt, :], b_sb[:], op=ALU.subtract
                    )
                    nc.vector.tensor_scalar_max(d_t[:], d_t[:], CLAMP)
                    p_t = work.tile([128, 512], F16, tag="p")
                    nc.scalar.activation(p_t[:], d_t[:], AF.Exp)
                    for c in range(4):
                        nc.tensor.matmul(
                            o_ps[:, c, :],
                            p_t[:, ts(c, 128)],
                            x_t,
                            start=(jt == 0),
                            stop=(jt == NJT - 1),
                        )
                    nc.tensor.matmul(
                        sum_ps[:],
                        ones_col[:],
                        p_t[:],
                        start=(jt == 0),
                        stop=(jt == NJT - 1),
                    )

            def b4_drain(h, o_ps, sum_ps):
                srow = pool.tile([1, 512], F32, tag="srow")
                nc.scalar.copy(srow[:], sum_ps[:])
                rec_row = pool.tile([1, 512], F32, tag="rec")
                nc.vector.reciprocal(rec_row[:], srow[:])
                rcol = pool.tile([128, 4], F32, tag="rcol")
                for c in range(4):
                    rc_ps = ps_qk.tile([128, 128], F32, tag="qk")
                    nc.tensor.transpose(
                        rc_ps[:, 0:1], rec_row[:, ts(c, 128)], one_one[:]
                    )
                    nc.vector.tensor_copy(rcol[:, c : c + 1], rc_ps[:, 0:1])
                for c in range(4):
                    o_sb = osbp.tile([128, 512], F32, tag="osb")
                    nc.scalar.activation(
                        o_sb[:], o_ps[:, c, :], AF.Copy,
                        bias=0.0, scale=rcol[:, c : c + 1],
                    )
                    nc.sync.dma_start(o[ts(h * 4 + c, 128), :], o_sb[:])

            for h in range(NIH):
                st = big.tile([128, NJT, 512], F32, tag="big")
                mx = pool.tile([128, 512], F32, tag="mx")
                b1_qk(h, st, mx)
                b_sb = finalize_max(mx)
                o_ps = ps_o.tile([128, 4, 512], F32, tag="o")
                sum_ps = ps_sum.tile([1, 512], F32, tag="sum")
                b3_exp_and_accum(st, b_sb, o_ps, sum_ps)
                b4_drain(h, o_ps, sum_ps)

    nc.compile()
    return nc


def _get_nc():
    global _NC_CACHE
    if _NC_CACHE is None:
        _NC_CACHE = _build_nc()
    return _NC_CACHE


def kernel(rotation_params, entangle_params, inputs, _trace=False, _trace_kwargs=None):
    X = np.ascontiguousarray(inputs, dtype=np.float32)
    Wq = np.ascontiguousarray(rotation_params, dtype=np.float32)
    Wk = np.ascontiguousarray(entangle_params, dtype=np.float32)
    XT = np.ascontiguousarray(X.T)
    # blocked layouts: [p, blk, c, j] with 8KiB (f32) / 4KiB (f16) runs/partition
    XTB = np.ascontiguousarray(
        XT.reshape(4, 128, 16, 512).transpose(1, 2, 0, 3)
    )
    X16B = np.ascontiguousarray(
        X.astype(np.float16).reshape(16, 4, 128, 512).transpose(2, 0, 1, 3)
    )

    in_maps = []
    for c in range(NCORES):
        in_maps.append(
            {
                "xt": XTB,
                "xtm": np.ascontiguousarray(XT[:, c * MY_N : (c + 1) * MY_N]),
                "x": X16B,
                "wq": Wq,
                "wkt": np.ascontiguousarray(Wk.T),
            }
        )

    nc = _get_nc()
    kw = {}
    if _trace:
        kw["trace"] = True
        kw.update(_trace_kwargs or {})
    br = run_bass_kernel_spmd(nc, in_maps, core_ids=list(range(NCORES)), **kw)
    out = np.concatenate([r["o"] for r in br.results], axis=0)
    if _trace:
        return out, br
    return out



# revision 2
# speedup vs baseline: 1.5265x; 1.5265x over previous
"""Self-attention kernel for Trainium2 (8 NeuronCores, SPMD).

Problem: X[8192,512], Wq,Wk[512,512]:
    Q = X@Wq ; K = X@Wk ; S = softmax(Q K^T / sqrt(512)) ; out = S @ X

Sharding: rows of Q (query blocks of 1024) across 8 cores; K/V (=X) replicated.

Per-core dataflow (core owns query rows i in [c*1024, (c+1)*1024)):
  Phase P:  M^T = Wq Wk^T            [512,512]  (16 MMs, f32r)
            R   = (M X_mine^T)/sqrt(d) [512,1024] (32 MMs; S^T = X R)
  Per i-half h (512 query columns):
    B1: for each j-tile (64): S^T tile [j=128, i=512] = sum_e X^T_tile.T R
        (4 accumulating f32r matmuls) -> ACT copy PSUM->SBUF (S region),
        DVE running elementwise max -> mx[128,512]
    B2: partition-reduce mx via PE transpose + DVE reduce_max -> m[1,512];
        broadcast back to [128,512] via ones outer-product matmul
    B3: for each j-tile: d = S^T_t - B (DVE), exp (ACT, f16 out) -> P~;
        PE per i-chunk c: o[c,512] += P~[:,c].T @ X[j-tile] (N=512 fp16 MM)
        followed by a free N=1 MM on the same stationary:
        sum[c-chunk i,1] += P~[:,c].T @ ones  (row sums, already transposed)
    B4: DVE recip(sum[128,4]) -> ACT Copy-with-per-partition-scale drains
        o PSUM -> SBUF, DMA out.  (No transposes needed in the tail.)

The host supplies X^T blocked, the per-core X^T slice, X blocked fp16 and
W^T matrices (layout staging only; all FLOPs happen on device). fp32r
matmuls keep ~13 mantissa bits => logit noise ~0.08 => out rel err ~6e-3.
"""
import sys

sys.path.insert(0, "/opt/trn_rl_repo")

import numpy as np

import concourse.bass as bass
import concourse.mybir as mybir
import concourse.tile as tile
from concourse import bacc
from concourse.bass import ts
from concourse.bass_utils import run_bass_kernel_spmd
from concourse.masks import make_identity

F32 = mybir.dt.float32
F32R = mybir.dt.float32r
F16 = mybir.dt.float16
AF = mybir.ActivationFunctionType
ALU = mybir.AluOpType

N = 8192
D = 512
NCORES = 8
MY_N = N // NCORES          # 1024 query rows per core
NJT = N // 128              # 64 j-tiles
NIH = MY_N // 512           # 2 i-halves
NBLK = N // 512             # 16 xt/x blocks

_NC_CACHE = None


def _build_nc():
    nc = bacc.Bacc(None, target_bir_lowering=False)

    xt = nc.dram_tensor("xt", [128, NBLK, 4, 512], F32R, kind="ExternalInput")  # X^T blocked
    xtm = nc.dram_tensor("xtm", [D, MY_N], F32R, kind="ExternalInput")   # X^T slice
    x = nc.dram_tensor("x", [128, NBLK, 4, 512], F16, kind="ExternalInput")  # X blocked fp16
    wqt = nc.dram_tensor("wqt", [D, D], F32R, kind="ExternalInput")      # Wq^T
    wkt = nc.dram_tensor("wkt", [D, D], F32R, kind="ExternalInput")      # Wk^T
    o = nc.dram_tensor("o", [MY_N, D], F32, kind="ExternalOutput")

    with tile.TileContext(nc) as tc:
        with (
            tc.tile_pool(name="pool", bufs=1) as pool,          # persistent
            tc.tile_pool(name="stream", bufs=3) as stream,      # w/mt then xt blocks
            tc.tile_pool(name="big", bufs=1) as big,            # xtm then S-region
            tc.tile_pool(name="rpool", bufs=1) as rpool,        # R
            tc.tile_pool(name="xs", bufs=3) as xsp,             # X tiles (B3)
            tc.tile_pool(name="workd", bufs=3) as workd,        # d
            tc.tile_pool(name="workp", bufs=3) as workp,        # p
            tc.tile_pool(name="osbp", bufs=2) as osbp,
            tc.tile_pool(name="ps_qk", bufs=3, space="PSUM") as ps_qk,
            tc.tile_pool(name="ps_o", bufs=1, space="PSUM") as ps_o,
            tc.tile_pool(name="ps_sum", bufs=1, space="PSUM") as ps_sum,
        ):
            # ---- constants ----
            ident = pool.tile([128, 128], F32)
            make_identity(nc, ident[:])
            ones_f32 = pool.tile([128, 2], F32)
            nc.vector.memset(ones_f32[:], 1.0)
            ones_col = pool.tile([128, 1], F16)    # rhs for row sums
            nc.vector.tensor_copy(ones_col[:], ones_f32[:, 0:1])
            ones_row_f32 = pool.tile([1, 128], F32)
            nc.vector.memset(ones_row_f32[:], 1.0)
            ones_row = pool.tile([1, 128], F32R)   # lhsT for broadcast
            nc.vector.tensor_copy(ones_row[:], ones_row_f32[:])

            # ---- inputs staged early (DMA order: weights, xtm, xt prefetch) ----
            wqt_sb = stream.tile([128, 4, 512], F32R, tag="stream")
            nc.sync.dma_start(wqt_sb[:], wqt[:].rearrange("(c p) f -> p c f", p=128))
            wkt_sb = stream.tile([128, 4, 512], F32R, tag="stream")
            nc.sync.dma_start(wkt_sb[:], wkt[:].rearrange("(c p) e -> p c e", p=128))
            xtm_sb = big.tile([128, 4, MY_N], F32R, tag="big")
            for ih in range(NIH):
                nc.sync.dma_start(
                    xtm_sb[:, :, ts(ih, 512)],
                    xtm[:, ts(ih, 512)].rearrange("(c p) i -> p c i", p=128),
                )

            # ---- Phase P1: M^T = Wq Wk^T (scaled by 1/sqrt(D)) ----
            scale = 1.0 / float(np.sqrt(D))
            mt_sb = stream.tile([128, 4, 512], F32R, tag="stream")
            for fc in range(4):
                mt_ps = ps_qk.tile([128, 512], F32, tag="qk")
                for dch in range(4):
                    nc.tensor.matmul(
                        mt_ps[:],
                        wqt_sb[:, dch, ts(fc, 128)],
                        wkt_sb[:, dch, :],
                        start=(dch == 0),
                        stop=(dch == 3),
                    )
                nc.scalar.activation(
                    mt_sb[:, fc, :], mt_ps[:], AF.Copy, bias=0.0, scale=scale
                )

            # ---- Phase P2: R = M X_mine^T  (so S^T tiles = X^T-chunk.T @ R) ----
            r_sb = rpool.tile([128, 4, MY_N], F32R, tag="r")
            for ih in range(NIH):
                for ech in range(4):
                    r_ps = ps_qk.tile([128, 512], F32, tag="qk")
                    for fch in range(4):
                        nc.tensor.matmul(
                            r_ps[:],
                            mt_sb[:, fch, ts(ech, 128)],
                            xtm_sb[:, fch, ts(ih, 512)],
                            start=(fch == 0),
                            stop=(fch == 3),
                        )
                    nc.scalar.copy(r_sb[:, ech, ts(ih, 512)], r_ps[:])

            def running_max(mx, s_ps, first):
                if first:
                    nc.vector.tensor_copy(mx[:], s_ps[:])
                else:
                    nc.vector.tensor_tensor(mx[:], mx[:], s_ps[:], op=ALU.max)

            def finalize_max(mx):
                """mx[128,512] -> b_sb[128,512] broadcast of per-i max."""
                mcol = pool.tile([128, 4], F32, tag="mcol")
                for c in range(4):
                    mt_ps = ps_qk.tile([128, 128], F32, tag="qk")
                    nc.tensor.transpose(mt_ps[:], mx[:, ts(c, 128)], ident[:])
                    nc.vector.reduce_max(
                        mcol[:, c : c + 1], mt_ps[:], axis=mybir.AxisListType.X
                    )
                mrow_ps = ps_qk.tile([1, 512], F32, tag="qk")
                for c in range(4):
                    nc.tensor.transpose(
                        mrow_ps[:, ts(c, 128)], mcol[:, c : c + 1], ident[:]
                    )
                mrow = pool.tile([1, 512], F32R, tag="mrow")
                nc.scalar.copy(mrow[:], mrow_ps[:])
                b_ps = ps_qk.tile([128, 512], F32, tag="qk")
                nc.tensor.matmul(b_ps[:], ones_row[:], mrow[:], start=True, stop=True)
                b_sb = pool.tile([128, 512], F32, tag="bsb")
                nc.scalar.copy(b_sb[:], b_ps[:])
                return b_sb

            def b1_qk(h, st, mx):
                for blk in range(NBLK):
                    xt_blk = stream.tile([128, 4, 512], F32R, tag="stream")
                    nc.sync.dma_start(xt_blk[:], xt[:, blk, :, :])
                    for t in range(4):
                        jt = blk * 4 + t
                        s_ps = ps_qk.tile([128, 512], F32, tag="qk")
                        for e in range(4):
                            nc.tensor.matmul(
                                s_ps[:],
                                xt_blk[:, e, ts(t, 128)],
                                r_sb[:, e, ts(h, 512)],
                                start=(e == 0),
                                stop=(e == 3),
                            )
                        nc.scalar.copy(st[:, jt, :], s_ps[:])
                        running_max(mx, s_ps, jt == 0)

            def b3_exp_and_accum(st, b_sb, o_ps, sum_ps):
                for blk in range(NBLK):
                    x_blk = xsp.tile([128, 4, 512], F16, tag="x")
                    nc.sync.dma_start(x_blk[:], x[:, blk, :, :])
                    for t in range(4):
                        jt = blk * 4 + t
                        d_t = workd.tile([128, 512], F32, tag="d")
                        nc.vector.tensor_tensor(
                            d_t[:], st[:, jt, :], b_sb[:], op=ALU.subtract
                        )
                        p_t = workp.tile([128, 512], F16, tag="p")
                        nc.scalar.activation(p_t[:], d_t[:], AF.Exp)
                        for c in range(4):
                            nc.tensor.matmul(
                                o_ps[:, c, :],
                                p_t[:, ts(c, 128)],
                                x_blk[:, t, :],
                                start=(jt == 0),
                                stop=(jt == NJT - 1),
                            )
                            # free row-sum ride-along: same stationary, N=1
                            nc.tensor.matmul(
                                sum_ps[:, c : c + 1],
                                p_t[:, ts(c, 128)],
                                ones_col[:],
                                start=(jt == 0 and c == 0),
                                stop=(jt == NJT - 1 and c == 3),
                            )

            def b4_drain(h, o_ps, sum_ps):
                rec = pool.tile([128, 4], F32, tag="rec")
                nc.vector.reciprocal(rec[:], sum_ps[:])
                for c in range(4):
                    o_sb = osbp.tile([128, 512], F32, tag="osb")
                    nc.scalar.activation(
                        o_sb[:], o_ps[:, c, :], AF.Copy,
                        bias=0.0, scale=rec[:, c : c + 1],
                    )
                    nc.sync.dma_start(o[ts(h * 4 + c, 128), :], o_sb[:])

            for h in range(NIH):
                st = big.tile([128, NJT, 512], F32, tag="big")
                mx = pool.tile([128, 512], F32, tag="mx")
                b1_qk(h, st, mx)
                b_sb = finalize_max(mx)
                o_ps = ps_o.tile([128, 4, 512], F32, tag="o")
                sum_ps = ps_sum.tile([128, 4], F32, tag="sum")
                b3_exp_and_accum(st, b_sb, o_ps, sum_ps)
                b4_drain(h, o_ps, sum_ps)

    nc.compile()
    return nc


def _get_nc():
    global _NC_CACHE
    if _NC_CACHE is None:
        _NC_CACHE = _build_nc()
    return _NC_CACHE


def kernel(rotation_params, entangle_params, inputs, _trace=False, _trace_kwargs=None):
    X = np.ascontiguousarray(inputs, dtype=np.float32)
    Wq = np.ascontiguousarray(rotation_params, dtype=np.float32)
    Wk = np.ascontiguousarray(entangle_params, dtype=np.float32)
    XT = np.ascontiguousarray(X.T)
    # blocked layouts: [p, blk, c, j] with 8KiB (f32) / 4KiB (f16) runs/partition
    XTB = np.ascontiguousarray(
        XT.reshape(4, 128, 16, 512).transpose(1, 2, 0, 3)
    )
    X16B = np.ascontiguousarray(
        X.astype(np.float16).reshape(16, 4, 128, 512).transpose(2, 0, 1, 3)
    )

    in_maps = []
    for c in range(NCORES):
        in_maps.append(
            {
                "xt": XTB,
                "xtm": np.ascontiguousarray(XT[:, c * MY_N : (c + 1) * MY_N]),
                "x": X16B,
                "wqt": np.ascontiguousarray(Wq.T),
                "wkt": np.ascontiguousarray(Wk.T),
            }
        )

    nc = _get_nc()
    kw = {}
    if _trace:
        kw["trace"] = True
        kw.update(_trace_kwargs or {})
    br = run_bass_kernel_spmd(nc, in_maps, core_ids=list(range(NCORES)), **kw)
    out = np.concatenate([r["o"] for r in br.results], axis=0)
    if _trace:
        return out, br
    return out
